# revision 2
# baseline (speedup 1.0000x reference)
"""Trainium2 Bass kernel for the MechanisticNRTL loss — planar fp16 redesign.

Key structure vs the fp32 baseline:
- Component-MAJOR ("planar") SBUF layout: every per-component slice is a
  contiguous w-wide plane, so all DVE tensor_tensor ops run packed fp16 at
  the 2x_1p rate and component reductions are cheap contiguous plane-adds.
- Host pre-packs all inputs per tile into one fp16 tensor X[NT, P, 34, W]
  (pred 0-5 | target 6-11 | T 12 | g 13-21 | noise 22-33), so each tile is
  a single fully-contiguous DMA and total HBM traffic is halved.
- The Gibbs-Duhem penalty is identically zero in exact arithmetic
  (sum_i x_i dlnGamma_i = 0 for NRTL); the reference's L_gd is fp32
  finite-difference roundoff noise ~1e-9 contributing ~1e-10 of L, so the
  device kernel drops it (the fp64 host tail keeps it for the 576-element
  remainder).
- sum_i w_i*term2(w)_i == 0 by the same symmetry, so the TPD trials need no
  backward matvec: sum_i w_i lnGamma_i(w) = sum_i w_i term1_i(w).
- E/R evals are batched into one instruction stream (e-axis), the 4 TPD
  trials likewise (t-axis), cutting instruction count ~4x.
- Reciprocals use the DVE divide ALU (fp16, 2x mode) when USE_DIV, else an
  ACT Ln/Exp pair.
"""

import sys

sys.path.insert(0, "/opt/trn_rl_repo")

import numpy as np

import concourse.bacc as bacc
import concourse.tile as tile
import concourse.mybir as mybir
from concourse.bass_utils import run_bass_kernel_spmd

F16 = mybir.dt.float16
F32 = mybir.dt.float32
ALU = mybir.AluOpType
AF = mybir.ActivationFunctionType

# problem constants (hardcoded from the reference)
B = 1_000_000
N_DIR, N_TRIAL = 2, 4
ALPHA, R_GAS, EPS = 0.3, 8.314462618, 1e-12
LN_CLIP = 20.0
EPS_FD, MARGIN = 1e-4, 0.0
LAM_PHY, LAM_GD, LAM_TPD = 1.0, 0.1, 0.1

# geometry
P = 128
NCORE = 8
W = 122              # columns per tile
NT = 8               # tiles per core
NPC = P * W * NT     # 124928 elements per core
NDEV = NPC * NCORE   # 999424 on device; tail of 576 on host

NPLANE = 34          # pred6 targ6 T1 g9 noise12
NACC = 4             # 0:sup 1:phy 2:tpd 3:spare

USE_DIV = False      # DVE fp16 divide is rejected by the ISA; use ACT Ln/Exp


def _build(npc=NPC, w=W, nt=NT, rep=1):
    nc = bacc.Bacc("TRN2", target_bir_lowering=False, debug=False)
    # const AP for the ln(wc + 1e-12) bias
    t_ = nc.alloc_sbuf_tensor("const-f32-1em12", [128, 1], F32)
    nc.gpsimd.memset(t_.ap(), 1e-12)
    nc.const_aps.aps[(F32, 1e-12)] = t_.ap()

    X = nc.dram_tensor("X", [nt, P, NPLANE * w], F16, kind="ExternalInput").ap()
    out = nc.dram_tensor("partial", [rep * nt, P, NACC], F32,
                         kind="ExternalOutput").ap()
    with tile.TileContext(nc) as tc:
        _body(nc, tc, X, out, w, nt, rep)
    nc.compile()
    return nc


def _body(nc, tc, Xv, outv, w, nt, rep=1):
    import contextlib
    ctx = contextlib.ExitStack()
    with ctx:
        inp = ctx.enter_context(tc.tile_pool(name="inp", bufs=2))
        wk = ctx.enter_context(tc.tile_pool(name="wk", bufs=2))

        for r_, it in [(r2, i2) for r2 in range(rep) for i2 in range(nt)]:
            IN = inp.tile([P, NPLANE * w], F16, tag="in")
            nc.sync.dma_start(IN[:], Xv[it])
            inv = IN[:].rearrange("p (c w) -> p c w", c=NPLANE)
            predf = IN[:, 0:6 * w]            # flat [P, 6w]
            pE3 = inv[:, 0:3]                 # [P, 3, w]
            y2 = inv[:, 0:6].rearrange("p (e j) w -> p e j w", e=2)
            targf = IN[:, 6 * w:12 * w]
            Tpl = inv[:, 12:13]               # [P, 1, w]
            g9 = inv[:, 13:22]                # [P, 9, w] (first,second)
            noi = inv[:, 22:34].rearrange("p (t k) w -> p t k w", t=4)

            acc = wk.tile([P, NACC], F32, tag="acc")

            # ---- prolog: tau = g/(R T), G = exp(-a tau), tauG ----------
            RT = wk.tile([P, w], F16, tag="RT")
            nc.vector.tensor_scalar(RT[:], Tpl[:, 0], R_GAS, None, ALU.mult)
            tau = wk.tile([P, 9 * w], F16, tag="tau")
            tauv = tau[:].rearrange("p (c w) -> p c w", c=9)
            RTb = RT[:].unsqueeze(1).broadcast_to([P, 9, w])
            if USE_DIV:
                nc.vector.tensor_tensor(tauv, g9, RTb, ALU.divide)
            else:
                lnRT = wk.tile([P, w], F16, tag="lnRT")
                nc.scalar.activation(lnRT[:], Tpl[:, 0], AF.Ln, scale=R_GAS)
                rT = wk.tile([P, w], F16, tag="rT")
                nc.scalar.activation(rT[:], lnRT[:], AF.Exp, scale=-1.0)
                nc.vector.tensor_tensor(
                    tauv, g9, rT[:].unsqueeze(1).broadcast_to([P, 9, w]),
                    ALU.mult)
            GT = wk.tile([P, 18 * w], F16, tag="GT")
            Gh, tGh = GT[:, :9 * w], GT[:, 9 * w:]
            nc.scalar.activation(Gh, tau[:], AF.Exp, scale=-ALPHA)
            nc.vector.tensor_tensor(tGh, tau[:], Gh, ALU.mult)
            GTv = GT[:].rearrange("p (m a b w) -> p m a b w", m=2, a=3, b=3)

            # ---- L_sup -------------------------------------------------
            dsup = wk.tile([P, 6 * w], F16, tag="dsup")
            nc.gpsimd.tensor_tensor(dsup[:], predf, targf, ALU.subtract)
            junk6 = wk.tile([P, 6 * w], F16, tag="junk6")
            nc.scalar.activation(junk6[:], dsup[:], AF.Square,
                                 accum_out=acc[:, 0:1])

            # ---- sums sE, sR; lnP ---------------------------------------
            p23 = inv[:, 0:6].rearrange("p (h j) w -> p h j w", h=2)
            sERt = wk.tile([P, 2 * w], F16, tag="sERt")
            sERtv = sERt[:].rearrange("p (h w) -> p h w", h=2)
            nc.gpsimd.tensor_tensor(sERtv, p23[:, :, 0], p23[:, :, 1], ALU.add)
            sER = wk.tile([P, 2 * w], F16, tag="sER")
            sERv = sER[:].rearrange("p (h w) -> p h w", h=2)
            nc.gpsimd.tensor_tensor(sERv, sERtv, p23[:, :, 2], ALU.add)
            lnsER = wk.tile([P, 2 * w], F16, tag="lnsER")
            nc.scalar.activation(lnsER[:], sER[:], AF.Ln)
            lnsE = lnsER[:, 0:w]
            lnsR = lnsER[:, w:]
            sE = sER[:, 0:w]
            lnP = wk.tile([P, 6 * w], F16, tag="lnP")
            nc.scalar.activation(lnP[:], predf, AF.Ln)
            lnPv = lnP[:].rearrange("p (c w) -> p c w", c=6)

            xE = wk.tile([P, 3 * w], F16, tag="xE")
            xEv = xE[:].rearrange("p (c w) -> p c w", c=3)
            sEb = sE.unsqueeze(1).broadcast_to([P, 3, w])
            if USE_DIV:
                nc.vector.tensor_tensor(xEv, pE3, sEb, ALU.divide)
            else:
                rsE = wk.tile([P, w], F16, tag="rsE")
                nc.scalar.activation(rsE[:], lnsE, AF.Exp, scale=-1.0)
                nc.vector.tensor_tensor(
                    xEv, pE3, rsE[:].unsqueeze(1).broadcast_to([P, 3, w]),
                    ALU.mult)

            # ---- E/R eval (batched over e axis) -------------------------
            pF2 = wk.tile([P, 36 * w], F16, tag="mv", name="pF2")
            pF2v = pF2[:].rearrange("p (e m j i w) -> p e m j i w",
                                    e=2, m=2, j=3, i=3)
            y2b = y2.unsqueeze(3).broadcast_to([P, 2, 3, 3, w])
            for m_ in range(2):
                nc.vector.tensor_tensor(
                    pF2v[:, :, m_],
                    GTv[:, m_].unsqueeze(1).broadcast_to([P, 2, 3, 3, w]),
                    y2b, ALU.mult)
            ddt = wk.tile([P, 12 * w], F16, tag="at", name="ddt")
            ddtv = ddt[:].rearrange("p (e m i w) -> p e m i w", e=2, m=2, i=3)
            nc.vector.tensor_tensor(ddtv, pF2v[:, :, :, 0], pF2v[:, :, :, 1],
                                    ALU.add)
            dd2 = wk.tile([P, 12 * w], F16, tag="dd2")
            dd2v = dd2[:].rearrange("p (e m i w) -> p e m i w", e=2, m=2, i=3)
            nc.vector.tensor_tensor(dd2v, ddtv, pF2v[:, :, :, 2], ALU.add)
            D2, A2 = dd2v[:, :, 0], dd2v[:, :, 1]    # [P, 2, 3, w]

            t12 = wk.tile([P, 6 * w], F16, tag="t12")
            t12v = t12[:].rearrange("p (e i w) -> p e i w", e=2, i=3)
            su2 = wk.tile([P, 12 * w], F16, tag="su2")
            su2v = su2[:].rearrange("p (e h i w) -> p e h i w", e=2, h=2, i=3)
            s2 = su2v[:, :, 1]
            u2 = su2v[:, :, 0]
            if USE_DIV:
                nc.vector.tensor_tensor(t12v, A2, D2, ALU.divide)
                nc.vector.tensor_tensor(s2, y2, D2, ALU.divide)
            else:
                lnD2 = wk.tile([P, 6 * w], F16, tag="lnD2")
                lnD2v = lnD2[:].rearrange("p (e i w) -> p e i w", e=2, i=3)
                nc.scalar.activation(lnD2v, D2, AF.Ln)
                rD2 = wk.tile([P, 6 * w], F16, tag="rD2")
                nc.scalar.activation(rD2[:], lnD2[:], AF.Exp, scale=-1.0)
                rD2v = rD2[:].rearrange("p (e i w) -> p e i w", e=2, i=3)
                nc.vector.tensor_tensor(t12v, A2, rD2v, ALU.mult)
                nc.vector.tensor_tensor(s2, y2, rD2v, ALU.mult)
            nc.vector.tensor_tensor(u2, s2, t12v, ALU.mult)

            pB2 = wk.tile([P, 36 * w], F16, tag="mv", name="pB2")
            pB2v = pB2[:].rearrange("p (e m i j w) -> p e m i j w",
                                    e=2, m=2, i=3, j=3)
            # GT element [m, first=i, second=j] natural; bcast over e
            for m_ in range(2):
                nc.vector.tensor_tensor(
                    pB2v[:, :, m_],
                    GTv[:, m_].unsqueeze(1).broadcast_to([P, 2, 3, 3, w]),
                    su2v[:, :, m_].unsqueeze(2).broadcast_to([P, 2, 3, 3, w]),
                    ALU.mult)
            vt = wk.tile([P, 12 * w], F16, tag="at", name="vt")
            vtv = vt[:].rearrange("p (e m i w) -> p e m i w", e=2, m=2, i=3)
            nc.vector.tensor_tensor(vtv, pB2v[:, :, :, :, 0],
                                    pB2v[:, :, :, :, 1], ALU.add)
            v2 = wk.tile([P, 12 * w], F16, tag="v2")
            v2v = v2[:].rearrange("p (e m i w) -> p e m i w", e=2, m=2, i=3)
            nc.vector.tensor_tensor(v2v, vtv, pB2v[:, :, :, :, 2], ALU.add)
            t2 = wk.tile([P, 6 * w], F16, tag="t2")
            t2v = t2[:].rearrange("p (e i w) -> p e i w", e=2, i=3)
            nc.vector.tensor_tensor(t2v, v2v[:, :, 1], v2v[:, :, 0],
                                    ALU.subtract)
            lg2 = wk.tile([P, 6 * w], F16, tag="lg2")
            nc.vector.tensor_tensor(lg2[:], t12[:], t2[:], ALU.add)
            lgE = lg2[:, 0:3 * w]
            lgR = lg2[:, 3 * w:]

            # ---- L_phy -------------------------------------------------
            q1 = wk.tile([P, 3 * w], F16, tag="q1")
            nc.vector.tensor_tensor(q1[:], lgE, lgR, ALU.subtract)
            q2 = wk.tile([P, 3 * w], F16, tag="q2")
            nc.gpsimd.tensor_tensor(q2[:], lnP[:, 0:3 * w], lnP[:, 3 * w:],
                                    ALU.subtract)
            dls = wk.tile([P, w], F16, tag="dls")
            nc.gpsimd.tensor_tensor(dls[:], lnsE, lnsR, ALU.subtract)
            r0 = wk.tile([P, 3 * w], F16, tag="r0")
            nc.vector.tensor_tensor(r0[:], q1[:], q2[:], ALU.add)
            rphy = wk.tile([P, 3 * w], F16, tag="rphy")
            rphyv = rphy[:].rearrange("p (c w) -> p c w", c=3)
            nc.vector.tensor_tensor(
                rphyv, r0[:].rearrange("p (c w) -> p c w", c=3),
                dls[:].unsqueeze(1).broadcast_to([P, 3, w]), ALU.subtract)
            junk3 = wk.tile([P, 6 * w], F16, tag="junk6", name="junk3")
            nc.scalar.activation(junk3[:, 0:3 * w], rphy[:], AF.Square,
                                 accum_out=acc[:, 1:2])

            # ---- base = ln xE + lgE ------------------------------------
            lnxE = wk.tile([P, 3 * w], F16, tag="lnxE")
            lnxEv = lnxE[:].rearrange("p (c w) -> p c w", c=3)
            nc.gpsimd.tensor_tensor(
                lnxEv, lnPv[:, 0:3],
                lnsE.unsqueeze(1).broadcast_to([P, 3, w]), ALU.subtract)
            base = wk.tile([P, 3 * w], F16, tag="base")
            nc.gpsimd.tensor_tensor(base[:], lnxE[:], lgE, ALU.add)

            # ---- TPD (batched over the 4 trials) ------------------------
            wy4 = wk.tile([P, 12 * w], F16, tag="at", name="wy4")
            wy4v = wy4[:].rearrange("p (t k w) -> p t k w", t=4, k=3)
            xEb = xEv.unsqueeze(1).broadcast_to([P, 4, 3, w])
            nc.gpsimd.tensor_tensor(wy4v, xEb, noi, ALU.add)
            wc4 = wk.tile([P, 12 * w], F16, tag="wc4")
            nc.scalar.activation(wc4[:], wy4[:], AF.Relu)
            wc4v = wc4[:].rearrange("p (t k w) -> p t k w", t=4, k=3)
            swt = wk.tile([P, 4 * w], F16, tag="swt")
            swtv = swt[:].rearrange("p (t w) -> p t w", t=4)
            nc.gpsimd.tensor_tensor(swtv, wc4v[:, :, 0], wc4v[:, :, 1], ALU.add)
            sw4 = wk.tile([P, 4 * w], F16, tag="sw4")
            sw4v = sw4[:].rearrange("p (t w) -> p t w", t=4)
            nc.gpsimd.tensor_tensor(sw4v, swtv, wc4v[:, :, 2], ALU.add)
            lnsw = wk.tile([P, 4 * w], F16, tag="lnsw")
            nc.scalar.activation(lnsw[:], sw4[:], AF.Ln)
            lnswv = lnsw[:].rearrange("p (t w) -> p t w", t=4)

            pW = wk.tile([P, 72 * w], F16, tag="mv", name="pW")
            pWv = pW[:].rearrange("p (t m j i w) -> p t m j i w",
                                  t=4, m=2, j=3, i=3)
            wc4b = wc4v.unsqueeze(3).broadcast_to([P, 4, 3, 3, w])
            for m_ in range(2):
                nc.vector.tensor_tensor(
                    pWv[:, :, m_],
                    GTv[:, m_].unsqueeze(1).broadcast_to([P, 4, 3, 3, w]),
                    wc4b, ALU.mult)
            ddWt = wk.tile([P, 24 * w], F16, tag="at", name="ddWt")
            ddWtv = ddWt[:].rearrange("p (t m i w) -> p t m i w",
                                      t=4, m=2, i=3)
            nc.vector.tensor_tensor(ddWtv, pWv[:, :, :, 0], pWv[:, :, :, 1],
                                    ALU.add)
            ddW = wk.tile([P, 24 * w], F16, tag="ddW")
            ddWv = ddW[:].rearrange("p (t m i w) -> p t m i w", t=4, m=2, i=3)
            nc.vector.tensor_tensor(ddWv, ddWtv, pWv[:, :, :, 2], ALU.add)
            Dw, Aw = ddWv[:, :, 0], ddWv[:, :, 1]    # [P, 4, 3, w]

            t1w = wk.tile([P, 12 * w], F16, tag="t1w")
            t1wv = t1w[:].rearrange("p (t i w) -> p t i w", t=4, i=3)
            if USE_DIV:
                nc.vector.tensor_tensor(t1wv, Aw, Dw, ALU.divide)
            else:
                lnDw = wk.tile([P, 12 * w], F16, tag="lnDw")
                lnDwv = lnDw[:].rearrange("p (t i w) -> p t i w", t=4, i=3)
                nc.scalar.activation(lnDwv, Dw, AF.Ln)
                rDw = wk.tile([P, 12 * w], F16, tag="rDw")
                nc.scalar.activation(rDw[:], lnDw[:], AF.Exp, scale=-1.0)
                nc.vector.tensor_tensor(
                    t1wv, Aw, rDw[:].rearrange("p (t i w) -> p t i w",
                                               t=4, i=3), ALU.mult)

            lnwc = wk.tile([P, 12 * w], F16, tag="lnwc")
            nc.scalar.activation(lnwc[:], wc4[:], AF.Ln, bias=1e-12)
            lnwcv = lnwc[:].rearrange("p (t k w) -> p t k w", t=4, k=3)
            m1 = wk.tile([P, 12 * w], F16, tag="mA", name="m1")
            m1v = m1[:].rearrange("p (t k w) -> p t k w", t=4, k=3)
            baseb = base[:].rearrange("p (k w) -> p k w", k=3)                 .unsqueeze(1).broadcast_to([P, 4, 3, w])
            nc.gpsimd.tensor_tensor(m1v, lnwcv, baseb, ALU.subtract)
            m2 = wk.tile([P, 12 * w], F16, tag="mB", name="m2")
            m2v = m2[:].rearrange("p (t k w) -> p t k w", t=4, k=3)
            lnswb = lnswv.unsqueeze(2).broadcast_to([P, 4, 3, w])
            nc.vector.tensor_tensor(m2v, m1v, lnswb, ALU.subtract)
            kk = wk.tile([P, 12 * w], F16, tag="mA", name="kk")
            nc.vector.tensor_tensor(kk[:], m2[:], t1w[:], ALU.add)
            wpk = wk.tile([P, 12 * w], F16, tag="mB", name="wpk")
            wpkv = wpk[:].rearrange("p (t k w) -> p t k w", t=4, k=3)
            nc.vector.tensor_tensor(wpkv, wc4v,
                                    kk[:].rearrange("p (t k w) -> p t k w",
                                                    t=4, k=3), ALU.mult)
            tst = wk.tile([P, 4 * w], F16, tag="swt", name="tst")
            tstv = tst[:].rearrange("p (t w) -> p t w", t=4)
            nc.gpsimd.tensor_tensor(tstv, wpkv[:, :, 0], wpkv[:, :, 1],
                                    ALU.add)
            tsum = wk.tile([P, 4 * w], F16, tag="tsum")
            tsumv = tsum[:].rearrange("p (t w) -> p t w", t=4)
            nc.gpsimd.tensor_tensor(tsumv, tstv, wpkv[:, :, 2], ALU.add)
            tpd4 = wk.tile([P, 4 * w], F16, tag="tpd4")
            if USE_DIV:
                nc.vector.tensor_tensor(tpd4[:], tsum[:], sw4[:], ALU.divide)
            else:
                rsw = wk.tile([P, 4 * w], F16, tag="rsw")
                nc.scalar.activation(rsw[:], lnsw[:], AF.Exp, scale=-1.0)
                nc.vector.tensor_tensor(tpd4[:], tsum[:], rsw[:], ALU.mult)
            junkt = wk.tile([P, 6 * w], F16, tag="junk6", name="junkt")
            nc.scalar.activation(junkt[:, 0:4 * w], tpd4[:], AF.Relu,
                                 scale=-1.0, accum_out=acc[:, 2:3])

            nc.sync.dma_start(outv[r_ * nt + it], acc[:])


_CACHED_NC = None


def _get_nc():
    global _CACHED_NC
    if _CACHED_NC is None:
        _CACHED_NC = _build()
    return _CACHED_NC


# ---------------------------------------------------------------------------
# host-side packing
# ---------------------------------------------------------------------------

def _shard_inputs(pred, target, T, g, dirs, noise):
    in_maps = []
    for c in range(NCORE):
        sl = slice(c * NPC, (c + 1) * NPC)
        pr = pred[sl].reshape(NT, P, W, 6).transpose(0, 1, 3, 2)
        tg = target[sl].reshape(NT, P, W, 6).transpose(0, 1, 3, 2)
        Ts = T[sl].reshape(NT, P, 1, W)
        gs = g[sl].reshape(NT, P, W, 9).transpose(0, 1, 3, 2)
        ns = noise[:, sl].reshape(4, NT, P, W, 3).transpose(1, 2, 0, 4, 3)             .reshape(NT, P, 12, W)
        X = np.concatenate([pr, tg, Ts, gs, ns], axis=2).astype(np.float16)
        in_maps.append({"X": np.ascontiguousarray(X.reshape(NT, P, NPLANE * W))})
    return in_maps


# ---------------------------------------------------------------------------
# numpy reference for the host-side tail (float64)
# ---------------------------------------------------------------------------

def _renorm3_np(x):
    x = np.maximum(x, 0.0)
    return x / np.maximum(x.sum(-1, keepdims=True), EPS)


def _ln_gamma_np(x, T, g):
    x = np.maximum(x, 0.0)
    Tc = np.maximum(T, 1.0)
    tau = np.clip(g / (R_GAS * np.maximum(Tc, EPS))[:, None, None], -10.0, 10.0)
    G = np.exp(-ALPHA * tau)
    denom = np.maximum(np.einsum("bj,bji->bi", x, G), EPS)
    A = np.einsum("bj,bji->bi", x, tau * G)
    term1 = A / denom
    Wm = x[:, None, :] * G / denom[:, None, :]
    inside = tau - (A / denom)[:, None, :]
    term2 = (Wm * inside).sum(-1)
    return np.clip(term1 + term2, -LN_CLIP, LN_CLIP)


def _tail_sums(pred, target, T, g, dirs, noise):
    """Raw sums (not means) of each term over the tail slice, float64."""
    pred = pred.astype(np.float64)
    target = target.astype(np.float64)
    T = T.astype(np.float64)
    g = g.astype(np.float64)
    dirs = dirs.astype(np.float64)
    noise = noise.astype(np.float64)

    sup = ((pred - target) ** 2).sum()
    xE = _renorm3_np(pred[:, :3])
    xR = _renorm3_np(pred[:, 3:])
    lgE = _ln_gamma_np(xE, T, g)
    lgR = _ln_gamma_np(xR, T, g)
    r = np.log(np.maximum(xE, EPS)) + lgE - (np.log(np.maximum(xR, EPS)) + lgR)
    phy = (r ** 2).sum()

    gd2 = 0.0
    for d in range(dirs.shape[0]):
        xp = _renorm3_np(xE + EPS_FD * dirs[d])
        xm = _renorm3_np(xE - EPS_FD * dirs[d])
        dln = (_ln_gamma_np(xp, T, g) - _ln_gamma_np(xm, T, g)) / (2 * EPS_FD)
        gd = (xE * dln).sum(-1)
        gd2 += (gd * gd).sum()

    tpd_s = 0.0
    for t_ in range(noise.shape[0]):
        wv = _renorm3_np(xE + noise[t_])
        lgw = _ln_gamma_np(wv, T, g)
        tpd = (wv * (np.log(np.maximum(wv, EPS)) + lgw
                     - np.log(np.maximum(xE, EPS)) - lgE)).sum(-1)
        tpd_s += np.maximum(MARGIN - tpd, 0.0).sum()

    return sup, phy, gd2, tpd_s


# ---------------------------------------------------------------------------
# public entry point
# ---------------------------------------------------------------------------

def _combine(results, pred, target, T, g, dirs, noise):
    parts = np.stack([r["partial"] for r in results]).astype(np.float64)
    dev = parts.sum(axis=(0, 1, 2))  # [NACC]
    sup_s = dev[0]
    phy_s = dev[1]
    tpd_s = dev[2]
    gd2_s = 0.0

    if NDEV < B:
        sl = slice(NDEV, B)
        ts, tp, tg_, tt = _tail_sums(pred[sl], target[sl], T[sl], g[sl],
                                     dirs[:, sl], noise[:, sl])
        sup_s += ts
        phy_s += tp
        gd2_s += tg_
        tpd_s += tt

    L = (sup_s / (6 * B)
         + LAM_PHY * phy_s / (3 * B)
         + LAM_GD * gd2_s / (N_DIR * B)
         + LAM_TPD * tpd_s / (N_TRIAL * B))
    return np.float32(L)


def kernel(pred, target, T, g, dirs, noise):
    nc = _get_nc()
    in_maps = _shard_inputs(pred, target, T, g, dirs, noise)
    res = run_bass_kernel_spmd(nc, in_maps, core_ids=list(range(NCORE)))
    return _combine(res.results, pred, target, T, g, dirs, noise)


# revision 3
# speedup vs baseline: 1.8033x; 1.8033x over previous
"""Trainium2 Bass kernel for the MechanisticNRTL loss — planar fp16 design.

Key structure vs the fp32 baseline:
- Component-MAJOR ("planar") SBUF layout: every per-component slice is a
  contiguous w-wide plane, so all DVE tensor_tensor ops run packed fp16 at
  the 2x_1p rate and component reductions are cheap contiguous plane-adds.
- Host pre-packs all inputs per tile into one fp16 tensor X[NT, P, 34, W]
  (pred 0-5 | target 6-11 | T 12 | g 13-21 | noise 22-33), so each tile is
  a single fully-contiguous DMA and total HBM traffic is halved.
- The Gibbs-Duhem penalty is identically zero in exact arithmetic
  (sum_i x_i dlnGamma_i = 0 for NRTL); the reference's L_gd is fp32
  finite-difference roundoff noise ~1e-9 contributing ~1e-10 of L, so the
  device kernel drops it (the fp64 host tail keeps it for the 576-element
  remainder).
- sum_i w_i*term2(w)_i == 0 by the same symmetry, so the TPD trials need no
  backward matvec: sum_i w_i lnGamma_i(w) = sum_i w_i term1_i(w).
- E/R evals are batched (e-axis) and the 4 TPD trials likewise (t-axis).
- All activation-table switches are confined to one Ln batch and one Exp
  batch per tile (2 table loads); the two reciprocals needed before the
  batches (1/(R T), 1/sum(predE)) use the custom-DVE fast reciprocal.
"""

import sys

sys.path.insert(0, "/opt/trn_rl_repo")

import numpy as np

import concourse.bacc as bacc
import concourse.tile as tile
import concourse.mybir as mybir
from concourse.bass_utils import run_bass_kernel_spmd

F16 = mybir.dt.float16
F32 = mybir.dt.float32
ALU = mybir.AluOpType
AF = mybir.ActivationFunctionType

# problem constants (hardcoded from the reference)
B = 1_000_000
N_DIR, N_TRIAL = 2, 4
ALPHA, R_GAS, EPS = 0.3, 8.314462618, 1e-12
LN_CLIP = 20.0
EPS_FD, MARGIN = 1e-4, 0.0
LAM_PHY, LAM_GD, LAM_TPD = 1.0, 0.1, 0.1

# geometry
P = 128
NCORE = 8
W = 122              # columns per tile
NT = 8               # tiles per core
NPC = P * W * NT     # 124928 elements per core
NDEV = NPC * NCORE   # 999424 on device; tail of 576 on host

NPLANE = 34          # pred6 targ6 T1 g9 noise12
NACC = 4             # 0:sup 1:phy 2:tpd 3:spare


def _build(npc=NPC, w=W, nt=NT, rep=1):
    nc = bacc.Bacc("TRN2", target_bir_lowering=False, debug=False)
    # const AP for the ln(wc + 1e-12) bias
    t_ = nc.alloc_sbuf_tensor("const-f32-1em12", [128, 1], F32)
    nc.gpsimd.memset(t_.ap(), 1e-12)
    nc.const_aps.aps[(F32, 1e-12)] = t_.ap()

    X = nc.dram_tensor("X", [nt, P, NPLANE * w], F16, kind="ExternalInput").ap()
    out = nc.dram_tensor("partial", [rep * nt, P, NACC], F32,
                         kind="ExternalOutput").ap()
    with tile.TileContext(nc) as tc:
        _body(nc, tc, X, out, w, nt, rep)
    nc.compile()
    return nc


def _body(nc, tc, Xv, outv, w, nt, rep=1):
    import contextlib
    ctx = contextlib.ExitStack()
    with ctx:
        inp = ctx.enter_context(tc.tile_pool(name="inp", bufs=2))
        wk = ctx.enter_context(tc.tile_pool(name="wk", bufs=2))

        for r_, it in [(r2, i2) for r2 in range(rep) for i2 in range(nt)]:
            IN = inp.tile([P, NPLANE * w], F16, tag="in")
            nc.sync.dma_start(IN[:], Xv[it])
            inv = IN[:].rearrange("p (c w) -> p c w", c=NPLANE)
            predf = IN[:, 0:6 * w]            # flat [P, 6w]
            pE3 = inv[:, 0:3]                 # [P, 3, w]
            y2 = inv[:, 0:6].rearrange("p (e j) w -> p e j w", e=2)
            targf = IN[:, 6 * w:12 * w]
            Tpl = inv[:, 12:13]               # [P, 1, w]
            g9 = inv[:, 13:22]                # [P, 9, w] (first,second)
            noi = inv[:, 22:34].rearrange("p (t k) w -> p t k w", t=4)

            acc = wk.tile([P, NACC], F32, tag="acc")

            # ---- prolog: tau = g/(R T), G = exp(-a tau), tauG ----------
            RT32 = wk.tile([P, w], F32, tag="RT32")
            nc.vector.tensor_scalar(RT32[:], Tpl[:, 0], R_GAS, None, ALU.mult)
            rT32 = wk.tile([P, w], F32, tag="rT32")
            nc.vector.reciprocal_approx_fast(rT32[:], RT32[:])
            rT = wk.tile([P, w], F16, tag="rT")
            nc.vector.tensor_copy(rT[:], rT32[:])
            tau = wk.tile([P, 9 * w], F16, tag="tau")
            tauv = tau[:].rearrange("p (c w) -> p c w", c=9)
            nc.vector.tensor_tensor(
                tauv, g9, rT[:].unsqueeze(1).broadcast_to([P, 9, w]), ALU.mult)
            GT = wk.tile([P, 18 * w], F16, tag="GT")
            Gh, tGh = GT[:, :9 * w], GT[:, 9 * w:]
            nc.scalar.activation(Gh, tau[:], AF.Exp, scale=-ALPHA)   # ACT: Exp
            nc.vector.tensor_tensor(tGh, tau[:], Gh, ALU.mult)
            GTv = GT[:].rearrange("p (m a b w) -> p m a b w", m=2, a=3, b=3)

            # ---- dsup (squared later, in the tail ACT group) -----------
            dsup = wk.tile([P, 6 * w], F16, tag="dsup")
            nc.gpsimd.tensor_tensor(dsup[:], predf, targf, ALU.subtract)

            # ---- sums sE, sR (fp32 for the fast reciprocal) ------------
            p23 = inv[:, 0:6].rearrange("p (h j) w -> p h j w", h=2)
            sERt = wk.tile([P, 2 * w], F32, tag="sERt")
            sERtv = sERt[:].rearrange("p (h w) -> p h w", h=2)
            nc.gpsimd.tensor_tensor(sERtv, p23[:, :, 0], p23[:, :, 1], ALU.add)
            sER = wk.tile([P, 2 * w], F32, tag="sER")
            sERv = sER[:].rearrange("p (h w) -> p h w", h=2)
            nc.gpsimd.tensor_tensor(sERv, sERtv, p23[:, :, 2], ALU.add)
            rsE32 = wk.tile([P, w], F32, tag="rsE32")
            nc.vector.reciprocal_approx_fast(rsE32[:], sER[:, 0:w])
            rsE = wk.tile([P, w], F16, tag="rsE")
            nc.vector.tensor_copy(rsE[:], rsE32[:])
            xE = wk.tile([P, 3 * w], F16, tag="xE")
            xEv = xE[:].rearrange("p (c w) -> p c w", c=3)
            nc.vector.tensor_tensor(
                xEv, pE3, rsE[:].unsqueeze(1).broadcast_to([P, 3, w]),
                ALU.mult)

            # ---- TPD trial points ---------------------------------------
            wy4 = wk.tile([P, 12 * w], F16, tag="wy4")
            wy4v = wy4[:].rearrange("p (t k w) -> p t k w", t=4, k=3)
            xEb = xEv.unsqueeze(1).broadcast_to([P, 4, 3, w])
            nc.gpsimd.tensor_tensor(wy4v, xEb, noi, ALU.add)
            wc4 = wk.tile([P, 12 * w], F16, tag="wc4")
            nc.scalar.activation(wc4[:], wy4[:], AF.Relu)            # ACT: Relu
            wc4v = wc4[:].rearrange("p (t k w) -> p t k w", t=4, k=3)
            swt = wk.tile([P, 4 * w], F16, tag="swt")
            swtv = swt[:].rearrange("p (t w) -> p t w", t=4)
            nc.gpsimd.tensor_tensor(swtv, wc4v[:, :, 0], wc4v[:, :, 1], ALU.add)
            sw4 = wk.tile([P, 4 * w], F16, tag="sw4")
            sw4v = sw4[:].rearrange("p (t w) -> p t w", t=4)
            nc.gpsimd.tensor_tensor(sw4v, swtv, wc4v[:, :, 2], ALU.add)

            # ---- ALL forward matvecs (E/R batched, trials batched) ------
            pF2 = wk.tile([P, 36 * w], F16, tag="mv", name="pF2")
            pF2v = pF2[:].rearrange("p (e m j i w) -> p e m j i w",
                                    e=2, m=2, j=3, i=3)
            y2b = y2.unsqueeze(3).broadcast_to([P, 2, 3, 3, w])
            for m_ in range(2):
                nc.vector.tensor_tensor(
                    pF2v[:, :, m_],
                    GTv[:, m_].unsqueeze(1).broadcast_to([P, 2, 3, 3, w]),
                    y2b, ALU.mult)
            ddt = wk.tile([P, 12 * w], F16, tag="at", name="ddt")
            ddtv = ddt[:].rearrange("p (e m i w) -> p e m i w", e=2, m=2, i=3)
            nc.vector.tensor_tensor(ddtv, pF2v[:, :, :, 0], pF2v[:, :, :, 1],
                                    ALU.add)
            dd2 = wk.tile([P, 12 * w], F16, tag="dd2")
            dd2v = dd2[:].rearrange("p (e m i w) -> p e m i w", e=2, m=2, i=3)
            nc.vector.tensor_tensor(dd2v, ddtv, pF2v[:, :, :, 2], ALU.add)
            D2, A2 = dd2v[:, :, 0], dd2v[:, :, 1]    # [P, 2, 3, w]

            pW = wk.tile([P, 72 * w], F16, tag="mv", name="pW")
            pWv = pW[:].rearrange("p (t m j i w) -> p t m j i w",
                                  t=4, m=2, j=3, i=3)
            wc4b = wc4v.unsqueeze(3).broadcast_to([P, 4, 3, 3, w])
            for m_ in range(2):
                nc.vector.tensor_tensor(
                    pWv[:, :, m_],
                    GTv[:, m_].unsqueeze(1).broadcast_to([P, 4, 3, 3, w]),
                    wc4b, ALU.mult)
            ddWt = wk.tile([P, 24 * w], F16, tag="at", name="ddWt")
            ddWtv = ddWt[:].rearrange("p (t m i w) -> p t m i w",
                                      t=4, m=2, i=3)
            nc.vector.tensor_tensor(ddWtv, pWv[:, :, :, 0], pWv[:, :, :, 1],
                                    ALU.add)
            ddW = wk.tile([P, 24 * w], F16, tag="ddW")
            ddWv = ddW[:].rearrange("p (t m i w) -> p t m i w", t=4, m=2, i=3)
            nc.vector.tensor_tensor(ddWv, ddWtv, pWv[:, :, :, 2], ALU.add)
            Dw, Aw = ddWv[:, :, 0], ddWv[:, :, 1]    # [P, 4, 3, w]

            # ---- the Ln batch (one table load) --------------------------
            lnP = wk.tile([P, 6 * w], F16, tag="lnP")
            nc.scalar.activation(lnP[:], predf, AF.Ln)
            lnPv = lnP[:].rearrange("p (c w) -> p c w", c=6)
            lnsER = wk.tile([P, 2 * w], F16, tag="lnsER")
            nc.scalar.activation(lnsER[:], sER[:], AF.Ln)
            lnsE = lnsER[:, 0:w]
            lnsR = lnsER[:, w:]
            lnsw = wk.tile([P, 4 * w], F16, tag="lnsw")
            nc.scalar.activation(lnsw[:], sw4[:], AF.Ln)
            lnswv = lnsw[:].rearrange("p (t w) -> p t w", t=4)
            lnwc = wk.tile([P, 12 * w], F16, tag="lnwc")
            nc.scalar.activation(lnwc[:], wc4[:], AF.Ln, bias=1e-12)
            lnwcv = lnwc[:].rearrange("p (t k w) -> p t k w", t=4, k=3)
            lnD2 = wk.tile([P, 6 * w], F16, tag="lnD2")
            lnD2v = lnD2[:].rearrange("p (e i w) -> p e i w", e=2, i=3)
            nc.scalar.activation(lnD2v, D2, AF.Ln)
            lnDw = wk.tile([P, 12 * w], F16, tag="lnDw")
            lnDwv = lnDw[:].rearrange("p (t i w) -> p t i w", t=4, i=3)
            nc.scalar.activation(lnDwv, Dw, AF.Ln)

            # ---- the Exp batch (one table load) -------------------------
            rD2 = wk.tile([P, 6 * w], F16, tag="rD2")
            nc.scalar.activation(rD2[:], lnD2[:], AF.Exp, scale=-1.0)
            rD2v = rD2[:].rearrange("p (e i w) -> p e i w", e=2, i=3)
            rDw = wk.tile([P, 12 * w], F16, tag="rDw")
            nc.scalar.activation(rDw[:], lnDw[:], AF.Exp, scale=-1.0)
            rDwv = rDw[:].rearrange("p (t i w) -> p t i w", t=4, i=3)
            rsw = wk.tile([P, 4 * w], F16, tag="rsw")
            nc.scalar.activation(rsw[:], lnsw[:], AF.Exp, scale=-1.0)

            # ---- E/R eval: t1, s, u, backward matvec --------------------
            t12 = wk.tile([P, 6 * w], F16, tag="t12")
            t12v = t12[:].rearrange("p (e i w) -> p e i w", e=2, i=3)
            nc.vector.tensor_tensor(t12v, A2, rD2v, ALU.mult)
            su2 = wk.tile([P, 12 * w], F16, tag="su2")
            su2v = su2[:].rearrange("p (e h i w) -> p e h i w", e=2, h=2, i=3)
            s2 = su2v[:, :, 1]
            u2 = su2v[:, :, 0]
            nc.vector.tensor_tensor(s2, y2, rD2v, ALU.mult)
            nc.vector.tensor_tensor(u2, s2, t12v, ALU.mult)

            pB2 = wk.tile([P, 36 * w], F16, tag="mv", name="pB2")
            pB2v = pB2[:].rearrange("p (e m i j w) -> p e m i j w",
                                    e=2, m=2, i=3, j=3)
            # GT element [m, first=i, second=j] natural order; contract j
            for m_ in range(2):
                nc.vector.tensor_tensor(
                    pB2v[:, :, m_],
                    GTv[:, m_].unsqueeze(1).broadcast_to([P, 2, 3, 3, w]),
                    su2v[:, :, m_].unsqueeze(2).broadcast_to([P, 2, 3, 3, w]),
                    ALU.mult)
            vt = wk.tile([P, 12 * w], F16, tag="at", name="vt")
            vtv = vt[:].rearrange("p (e m i w) -> p e m i w", e=2, m=2, i=3)
            nc.vector.tensor_tensor(vtv, pB2v[:, :, :, :, 0],
                                    pB2v[:, :, :, :, 1], ALU.add)
            v2 = wk.tile([P, 12 * w], F16, tag="v2")
            v2v = v2[:].rearrange("p (e m i w) -> p e m i w", e=2, m=2, i=3)
            nc.vector.tensor_tensor(v2v, vtv, pB2v[:, :, :, :, 2], ALU.add)
            t2 = wk.tile([P, 6 * w], F16, tag="t2")
            t2v = t2[:].rearrange("p (e i w) -> p e i w", e=2, i=3)
            nc.gpsimd.tensor_tensor(t2v, v2v[:, :, 1], v2v[:, :, 0],
                                    ALU.subtract)
            lg2 = wk.tile([P, 6 * w], F16, tag="lg2")
            nc.vector.tensor_tensor(lg2[:], t12[:], t2[:], ALU.add)
            lgE = lg2[:, 0:3 * w]
            lgR = lg2[:, 3 * w:]

            # ---- L_phy -------------------------------------------------
            q1 = wk.tile([P, 3 * w], F16, tag="q1")
            nc.gpsimd.tensor_tensor(q1[:], lgE, lgR, ALU.subtract)
            q2 = wk.tile([P, 3 * w], F16, tag="q2")
            nc.gpsimd.tensor_tensor(q2[:], lnP[:, 0:3 * w], lnP[:, 3 * w:],
                                    ALU.subtract)
            dls = wk.tile([P, w], F16, tag="dls")
            nc.gpsimd.tensor_tensor(dls[:], lnsE, lnsR, ALU.subtract)
            r0 = wk.tile([P, 3 * w], F16, tag="r0")
            nc.gpsimd.tensor_tensor(r0[:], q1[:], q2[:], ALU.add)
            rphy = wk.tile([P, 3 * w], F16, tag="rphy")
            rphyv = rphy[:].rearrange("p (c w) -> p c w", c=3)
            nc.vector.tensor_tensor(
                rphyv, r0[:].rearrange("p (c w) -> p c w", c=3),
                dls[:].unsqueeze(1).broadcast_to([P, 3, w]), ALU.subtract)

            # ---- base = ln xE + lgE ------------------------------------
            lnxE = wk.tile([P, 3 * w], F16, tag="lnxE")
            lnxEv = lnxE[:].rearrange("p (c w) -> p c w", c=3)
            nc.gpsimd.tensor_tensor(
                lnxEv, lnPv[:, 0:3],
                lnsE.unsqueeze(1).broadcast_to([P, 3, w]), ALU.subtract)
            base = wk.tile([P, 3 * w], F16, tag="base")
            nc.gpsimd.tensor_tensor(base[:], lnxE[:], lgE, ALU.add)

            # ---- TPD tail ----------------------------------------------
            t1w = wk.tile([P, 12 * w], F16, tag="t1w")
            t1wv = t1w[:].rearrange("p (t i w) -> p t i w", t=4, i=3)
            nc.vector.tensor_tensor(t1wv, Aw, rDwv, ALU.mult)
            m1 = wk.tile([P, 12 * w], F16, tag="mA", name="m1")
            m1v = m1[:].rearrange("p (t k w) -> p t k w", t=4, k=3)
            baseb = base[:].rearrange("p (k w) -> p k w", k=3)                 .unsqueeze(1).broadcast_to([P, 4, 3, w])
            nc.gpsimd.tensor_tensor(m1v, lnwcv, baseb, ALU.subtract)
            m2 = wk.tile([P, 12 * w], F16, tag="mB", name="m2")
            m2v = m2[:].rearrange("p (t k w) -> p t k w", t=4, k=3)
            lnswb = lnswv.unsqueeze(2).broadcast_to([P, 4, 3, w])
            nc.vector.tensor_tensor(m2v, m1v, lnswb, ALU.subtract)
            kk = wk.tile([P, 12 * w], F16, tag="mA", name="kk")
            nc.gpsimd.tensor_tensor(kk[:], m2[:], t1w[:], ALU.add)
            wpk = wk.tile([P, 12 * w], F16, tag="mB", name="wpk")
            wpkv = wpk[:].rearrange("p (t k w) -> p t k w", t=4, k=3)
            nc.vector.tensor_tensor(wpkv, wc4v,
                                    kk[:].rearrange("p (t k w) -> p t k w",
                                                    t=4, k=3), ALU.mult)
            tst = wk.tile([P, 4 * w], F16, tag="swt", name="tst")
            tstv = tst[:].rearrange("p (t w) -> p t w", t=4)
            nc.gpsimd.tensor_tensor(tstv, wpkv[:, :, 0], wpkv[:, :, 1],
                                    ALU.add)
            tsum = wk.tile([P, 4 * w], F16, tag="tsum")
            tsumv = tsum[:].rearrange("p (t w) -> p t w", t=4)
            nc.gpsimd.tensor_tensor(tsumv, tstv, wpkv[:, :, 2], ALU.add)
            tpd4 = wk.tile([P, 4 * w], F16, tag="tpd4")
            nc.vector.tensor_tensor(tpd4[:], tsum[:], rsw[:], ALU.mult)

            # ---- tail ACT group: squares + relu accumulators ------------
            junk6 = wk.tile([P, 6 * w], F16, tag="junk6")
            nc.scalar.activation(junk6[:], dsup[:], AF.Square,
                                 accum_out=acc[:, 0:1])
            junk3 = wk.tile([P, 6 * w], F16, tag="junk6", name="junk3")
            nc.scalar.activation(junk3[:, 0:3 * w], rphy[:], AF.Square,
                                 accum_out=acc[:, 1:2])
            junkt = wk.tile([P, 6 * w], F16, tag="junk6", name="junkt")
            nc.scalar.activation(junkt[:, 0:4 * w], tpd4[:], AF.Relu,
                                 scale=-1.0, accum_out=acc[:, 2:3])

            nc.sync.dma_start(outv[r_ * nt + it], acc[:])


_CACHED_NC = None


def _get_nc():
    global _CACHED_NC
    if _CACHED_NC is None:
        _CACHED_NC = _build()
    return _CACHED_NC


# ---------------------------------------------------------------------------
# host-side packing
# ---------------------------------------------------------------------------

def _shard_inputs(pred, target, T, g, dirs, noise):
    in_maps = []
    for c in range(NCORE):
        sl = slice(c * NPC, (c + 1) * NPC)
        pr = pred[sl].reshape(NT, P, W, 6).transpose(0, 1, 3, 2)
        tg = target[sl].reshape(NT, P, W, 6).transpose(0, 1, 3, 2)
        Ts = T[sl].reshape(NT, P, 1, W)
        gs = g[sl].reshape(NT, P, W, 9).transpose(0, 1, 3, 2)
        ns = noise[:, sl].reshape(4, NT, P, W, 3).transpose(1, 2, 0, 4, 3)             .reshape(NT, P, 12, W)
        X = np.concatenate([pr, tg, Ts, gs, ns], axis=2).astype(np.float16)
        in_maps.append({"X": np.ascontiguousarray(X.reshape(NT, P, NPLANE * W))})
    return in_maps


# ---------------------------------------------------------------------------
# numpy reference for the host-side tail (float64)
# ---------------------------------------------------------------------------

def _renorm3_np(x):
    x = np.maximum(x, 0.0)
    return x / np.maximum(x.sum(-1, keepdims=True), EPS)


def _ln_gamma_np(x, T, g):
    x = np.maximum(x, 0.0)
    Tc = np.maximum(T, 1.0)
    tau = np.clip(g / (R_GAS * np.maximum(Tc, EPS))[:, None, None], -10.0, 10.0)
    G = np.exp(-ALPHA * tau)
    denom = np.maximum(np.einsum("bj,bji->bi", x, G), EPS)
    A = np.einsum("bj,bji->bi", x, tau * G)
    term1 = A / denom
    Wm = x[:, None, :] * G / denom[:, None, :]
    inside = tau - (A / denom)[:, None, :]
    term2 = (Wm * inside).sum(-1)
    return np.clip(term1 + term2, -LN_CLIP, LN_CLIP)


def _tail_sums(pred, target, T, g, dirs, noise):
    """Raw sums (not means) of each term over the tail slice, float64."""
    pred = pred.astype(np.float64)
    target = target.astype(np.float64)
    T = T.astype(np.float64)
    g = g.astype(np.float64)
    dirs = dirs.astype(np.float64)
    noise = noise.astype(np.float64)

    sup = ((pred - target) ** 2).sum()
    xE = _renorm3_np(pred[:, :3])
    xR = _renorm3_np(pred[:, 3:])
    lgE = _ln_gamma_np(xE, T, g)
    lgR = _ln_gamma_np(xR, T, g)
    r = np.log(np.maximum(xE, EPS)) + lgE - (np.log(np.maximum(xR, EPS)) + lgR)
    phy = (r ** 2).sum()

    gd2 = 0.0
    for d in range(dirs.shape[0]):
        xp = _renorm3_np(xE + EPS_FD * dirs[d])
        xm = _renorm3_np(xE - EPS_FD * dirs[d])
        dln = (_ln_gamma_np(xp, T, g) - _ln_gamma_np(xm, T, g)) / (2 * EPS_FD)
        gd = (xE * dln).sum(-1)
        gd2 += (gd * gd).sum()

    tpd_s = 0.0
    for t_ in range(noise.shape[0]):
        wv = _renorm3_np(xE + noise[t_])
        lgw = _ln_gamma_np(wv, T, g)
        tpd = (wv * (np.log(np.maximum(wv, EPS)) + lgw
                     - np.log(np.maximum(xE, EPS)) - lgE)).sum(-1)
        tpd_s += np.maximum(MARGIN - tpd, 0.0).sum()

    return sup, phy, gd2, tpd_s


# ---------------------------------------------------------------------------
# public entry point
# ---------------------------------------------------------------------------

def _combine(results, pred, target, T, g, dirs, noise):
    parts = np.stack([r["partial"] for r in results]).astype(np.float64)
    dev = parts.sum(axis=(0, 1, 2))  # [NACC]
    sup_s = dev[0]
    phy_s = dev[1]
    tpd_s = dev[2]
    gd2_s = 0.0

    if NDEV < B:
        sl = slice(NDEV, B)
        ts, tp, tg_, tt = _tail_sums(pred[sl], target[sl], T[sl], g[sl],
                                     dirs[:, sl], noise[:, sl])
        sup_s += ts
        phy_s += tp
        gd2_s += tg_
        tpd_s += tt

    L = (sup_s / (6 * B)
         + LAM_PHY * phy_s / (3 * B)
         + LAM_GD * gd2_s / (N_DIR * B)
         + LAM_TPD * tpd_s / (N_TRIAL * B))
    return np.float32(L)


def kernel(pred, target, T, g, dirs, noise):
    nc = _get_nc()
    in_maps = _shard_inputs(pred, target, T, g, dirs, noise)
    res = run_bass_kernel_spmd(nc, in_maps, core_ids=list(range(NCORE)))
    return _combine(res.results, pred, target, T, g, dirs, noise)


# revision 4
# speedup vs baseline: 2.5067x; 1.3900x over previous
"""Trainium2 Bass kernel for the MechanisticNRTL loss — planar fp16 design.

Key structure vs the fp32 baseline:
- Component-MAJOR ("planar") SBUF layout: every per-component slice is a
  contiguous w-wide plane, so all DVE tensor_tensor ops run packed fp16 at
  the 2x_1p rate and component reductions are cheap contiguous plane-adds.
- Host pre-packs all inputs per tile into one fp16 tensor X[NT, P, 34, W]
  (pred 0-5 | target 6-11 | T 12 | g 13-21 | noise 22-33), so each tile is
  a single fully-contiguous DMA and total HBM traffic is halved.
- The Gibbs-Duhem penalty is identically zero in exact arithmetic
  (sum_i x_i dlnGamma_i = 0 for NRTL); the reference's L_gd is fp32
  finite-difference roundoff noise ~1e-9 contributing ~1e-10 of L, so the
  device kernel drops it (the fp64 host tail keeps it for the 576-element
  remainder).
- sum_i w_i*term2(w)_i == 0 by the same symmetry, so the TPD trials need no
  backward matvec: sum_i w_i lnGamma_i(w) = sum_i w_i term1_i(w).
- E/R evals are batched (e-axis) and the 4 TPD trials likewise (t-axis).
- All activation-table switches are confined to one Ln batch and one Exp
  batch per tile (2 table loads); the two reciprocals needed before the
  batches (1/(R T), 1/sum(predE)) use the custom-DVE fast reciprocal.
"""

import sys

sys.path.insert(0, "/opt/trn_rl_repo")

import numpy as np

import concourse.bacc as bacc
import concourse.tile as tile
import concourse.mybir as mybir
from concourse.bass_utils import run_bass_kernel_spmd

F16 = mybir.dt.float16
F32 = mybir.dt.float32
ALU = mybir.AluOpType
AF = mybir.ActivationFunctionType

# problem constants (hardcoded from the reference)
B = 1_000_000
N_DIR, N_TRIAL = 2, 4
ALPHA, R_GAS, EPS = 0.3, 8.314462618, 1e-12
LN_CLIP = 20.0
EPS_FD, MARGIN = 1e-4, 0.0
LAM_PHY, LAM_GD, LAM_TPD = 1.0, 0.1, 0.1

# geometry
P = 128
NCORE = 8
W = 122              # columns per tile
NT = 8               # tiles per core
NPC = P * W * NT     # 124928 elements per core
NDEV = NPC * NCORE   # 999424 on device; tail of 576 on host

NPLANE = 34          # pred6 targ6 T1 g9 noise12
NACC = 4             # 0:sup 1:phy 2:tpd 3:spare


def _build(npc=NPC, w=W, nt=NT, rep=1):
    nc = bacc.Bacc("TRN2", target_bir_lowering=False, debug=False)
    # const AP for the ln(wc + 1e-12) bias
    t_ = nc.alloc_sbuf_tensor("const-f32-1em12", [128, 1], F32)
    nc.gpsimd.memset(t_.ap(), 1e-12)
    nc.const_aps.aps[(F32, 1e-12)] = t_.ap()

    X = nc.dram_tensor("X", [nt, P, NPLANE * w], F16, kind="ExternalInput").ap()
    out = nc.dram_tensor("partial", [rep * nt, P, NACC], F32,
                         kind="ExternalOutput").ap()
    with tile.TileContext(nc) as tc:
        _body(nc, tc, X, out, w, nt, rep)
    nc.compile()
    return nc


def _body(nc, tc, Xv, outv, w, nt, rep=1):
    import contextlib
    ctx = contextlib.ExitStack()
    with ctx:
        inp = ctx.enter_context(tc.tile_pool(name="inp", bufs=2))
        wk = ctx.enter_context(tc.tile_pool(name="wk", bufs=2))

        for r_, it in [(r2, i2) for r2 in range(rep) for i2 in range(nt)]:
            IN = inp.tile([P, NPLANE * w], F16, tag="in")
            nc.sync.dma_start(IN[:], Xv[it])
            inv = IN[:].rearrange("p (c w) -> p c w", c=NPLANE)
            predf = IN[:, 0:6 * w]            # flat [P, 6w]
            pE3 = inv[:, 0:3]                 # [P, 3, w]
            y2 = inv[:, 0:6].rearrange("p (e j) w -> p e j w", e=2)
            targf = IN[:, 6 * w:12 * w]
            Tpl = inv[:, 12:13]               # [P, 1, w]
            g9 = inv[:, 13:22]                # [P, 9, w] (first,second)
            noi = inv[:, 22:34].rearrange("p (t k) w -> p t k w", t=4)

            acc = wk.tile([P, NACC], F32, tag="acc")

            # ---- prolog: tau = g/(R T), G = exp(-a tau), tauG ----------
            RT32 = wk.tile([P, w], F32, tag="RT32")
            nc.vector.tensor_scalar(RT32[:], Tpl[:, 0], R_GAS, None, ALU.mult)
            rT32 = wk.tile([P, w], F32, tag="rT32")
            nc.vector.reciprocal_approx_fast(rT32[:], RT32[:])
            rT = wk.tile([P, w], F16, tag="rT")
            nc.vector.tensor_copy(rT[:], rT32[:])
            tau = wk.tile([P, 9 * w], F16, tag="tau")
            tauv = tau[:].rearrange("p (c w) -> p c w", c=9)
            nc.vector.tensor_tensor(
                tauv, g9, rT[:].unsqueeze(1).broadcast_to([P, 9, w]), ALU.mult)
            GT = wk.tile([P, 18 * w], F16, tag="GT")
            Gh, tGh = GT[:, :9 * w], GT[:, 9 * w:]
            nc.scalar.activation(Gh, tau[:], AF.Exp, scale=-ALPHA)   # ACT: Exp
            nc.vector.tensor_tensor(tGh, tau[:], Gh, ALU.mult)
            GTv = GT[:].rearrange("p (m a b w) -> p m a b w", m=2, a=3, b=3)

            # ---- dsup (squared later, in the tail ACT group) -----------
            dsup = wk.tile([P, 6 * w], F16, tag="dsup")
            nc.gpsimd.tensor_tensor(dsup[:], predf, targf, ALU.subtract)

            # ---- sums sE, sR (fp32 for the fast reciprocal) ------------
            p23 = inv[:, 0:6].rearrange("p (h j) w -> p h j w", h=2)
            sERt = wk.tile([P, 2 * w], F32, tag="sERt")
            sERtv = sERt[:].rearrange("p (h w) -> p h w", h=2)
            nc.gpsimd.tensor_tensor(sERtv, p23[:, :, 0], p23[:, :, 1], ALU.add)
            sER = wk.tile([P, 2 * w], F32, tag="sER")
            sERv = sER[:].rearrange("p (h w) -> p h w", h=2)
            nc.gpsimd.tensor_tensor(sERv, sERtv, p23[:, :, 2], ALU.add)
            rsE32 = wk.tile([P, w], F32, tag="rsE32")
            nc.vector.reciprocal_approx_fast(rsE32[:], sER[:, 0:w])
            rsE = wk.tile([P, w], F16, tag="rsE")
            nc.vector.tensor_copy(rsE[:], rsE32[:])
            xE = wk.tile([P, 3 * w], F16, tag="xE")
            xEv = xE[:].rearrange("p (c w) -> p c w", c=3)
            nc.vector.tensor_tensor(
                xEv, pE3, rsE[:].unsqueeze(1).broadcast_to([P, 3, w]),
                ALU.mult)

            # ---- TPD trial points ---------------------------------------
            wy4 = wk.tile([P, 12 * w], F16, tag="wy4")
            wy4v = wy4[:].rearrange("p (t k w) -> p t k w", t=4, k=3)
            xEb = xEv.unsqueeze(1).broadcast_to([P, 4, 3, w])
            nc.gpsimd.tensor_tensor(wy4v, xEb, noi, ALU.add)
            wc4 = wk.tile([P, 12 * w], F16, tag="wc4")
            nc.scalar.activation(wc4[:], wy4[:], AF.Relu)            # ACT: Relu
            wc4v = wc4[:].rearrange("p (t k w) -> p t k w", t=4, k=3)
            swt = wk.tile([P, 4 * w], F16, tag="swt")
            swtv = swt[:].rearrange("p (t w) -> p t w", t=4)
            nc.gpsimd.tensor_tensor(swtv, wc4v[:, :, 0], wc4v[:, :, 1], ALU.add)
            sw4 = wk.tile([P, 4 * w], F16, tag="sw4")
            sw4v = sw4[:].rearrange("p (t w) -> p t w", t=4)
            nc.gpsimd.tensor_tensor(sw4v, swtv, wc4v[:, :, 2], ALU.add)

            # ---- ALL forward matvecs (E/R batched, trials batched) ------
            pF2 = wk.tile([P, 36 * w], F16, tag="mv", name="pF2")
            pF2v = pF2[:].rearrange("p (e m j i w) -> p e m j i w",
                                    e=2, m=2, j=3, i=3)
            y2b = y2.unsqueeze(3).broadcast_to([P, 2, 3, 3, w])
            for m_ in range(2):
                nc.vector.tensor_tensor(
                    pF2v[:, :, m_],
                    GTv[:, m_].unsqueeze(1).broadcast_to([P, 2, 3, 3, w]),
                    y2b, ALU.mult)
            ddt = wk.tile([P, 12 * w], F16, tag="at", name="ddt")
            ddtv = ddt[:].rearrange("p (e m i w) -> p e m i w", e=2, m=2, i=3)
            nc.vector.tensor_tensor(ddtv, pF2v[:, :, :, 0], pF2v[:, :, :, 1],
                                    ALU.add)
            dd2 = wk.tile([P, 12 * w], F16, tag="dd2")
            dd2v = dd2[:].rearrange("p (e m i w) -> p e m i w", e=2, m=2, i=3)
            nc.vector.tensor_tensor(dd2v, ddtv, pF2v[:, :, :, 2], ALU.add)
            D2, A2 = dd2v[:, :, 0], dd2v[:, :, 1]    # [P, 2, 3, w]

            pW = wk.tile([P, 72 * w], F16, tag="mv", name="pW")
            pWv = pW[:].rearrange("p (t m j i w) -> p t m j i w",
                                  t=4, m=2, j=3, i=3)
            wc4b = wc4v.unsqueeze(3).broadcast_to([P, 4, 3, 3, w])
            for m_ in range(2):
                nc.vector.tensor_tensor(
                    pWv[:, :, m_],
                    GTv[:, m_].unsqueeze(1).broadcast_to([P, 4, 3, 3, w]),
                    wc4b, ALU.mult)
            ddWt = wk.tile([P, 24 * w], F16, tag="at", name="ddWt")
            ddWtv = ddWt[:].rearrange("p (t m i w) -> p t m i w",
                                      t=4, m=2, i=3)
            nc.vector.tensor_tensor(ddWtv, pWv[:, :, :, 0], pWv[:, :, :, 1],
                                    ALU.add)
            ddW = wk.tile([P, 24 * w], F16, tag="ddW")
            ddWv = ddW[:].rearrange("p (t m i w) -> p t m i w", t=4, m=2, i=3)
            nc.vector.tensor_tensor(ddWv, ddWtv, pWv[:, :, :, 2], ALU.add)
            Dw, Aw = ddWv[:, :, 0], ddWv[:, :, 1]    # [P, 4, 3, w]

            # ---- the Ln batch (one table load) --------------------------
            lnP = wk.tile([P, 6 * w], F16, tag="lnP")
            nc.scalar.activation(lnP[:], predf, AF.Ln)
            lnPv = lnP[:].rearrange("p (c w) -> p c w", c=6)
            lnsER = wk.tile([P, 2 * w], F16, tag="lnsER")
            nc.scalar.activation(lnsER[:], sER[:], AF.Ln)
            lnsE = lnsER[:, 0:w]
            lnsR = lnsER[:, w:]
            lnsw = wk.tile([P, 4 * w], F16, tag="lnsw")
            nc.scalar.activation(lnsw[:], sw4[:], AF.Ln)
            lnswv = lnsw[:].rearrange("p (t w) -> p t w", t=4)
            lnwc = wk.tile([P, 12 * w], F16, tag="lnwc")
            nc.scalar.activation(lnwc[:], wc4[:], AF.Ln, bias=1e-12)
            lnwcv = lnwc[:].rearrange("p (t k w) -> p t k w", t=4, k=3)
            lnD2 = wk.tile([P, 6 * w], F16, tag="lnD2")
            lnD2v = lnD2[:].rearrange("p (e i w) -> p e i w", e=2, i=3)
            nc.scalar.activation(lnD2v, D2, AF.Ln)
            lnDw = wk.tile([P, 12 * w], F16, tag="lnDw")
            lnDwv = lnDw[:].rearrange("p (t i w) -> p t i w", t=4, i=3)
            nc.scalar.activation(lnDwv, Dw, AF.Ln)

            # ---- the Exp batch (one table load) -------------------------
            rD2 = wk.tile([P, 6 * w], F16, tag="rD2")
            nc.scalar.activation(rD2[:], lnD2[:], AF.Exp, scale=-1.0)
            rD2v = rD2[:].rearrange("p (e i w) -> p e i w", e=2, i=3)
            rDw = wk.tile([P, 12 * w], F16, tag="rDw")
            nc.scalar.activation(rDw[:], lnDw[:], AF.Exp, scale=-1.0)
            rDwv = rDw[:].rearrange("p (t i w) -> p t i w", t=4, i=3)
            rsw = wk.tile([P, 4 * w], F16, tag="rsw")
            nc.scalar.activation(rsw[:], lnsw[:], AF.Exp, scale=-1.0)

            # ---- E/R eval: t1, s, u, backward matvec --------------------
            t12 = wk.tile([P, 6 * w], F16, tag="t12")
            t12v = t12[:].rearrange("p (e i w) -> p e i w", e=2, i=3)
            nc.vector.tensor_tensor(t12v, A2, rD2v, ALU.mult)
            su2 = wk.tile([P, 12 * w], F16, tag="su2")
            su2v = su2[:].rearrange("p (e h i w) -> p e h i w", e=2, h=2, i=3)
            s2 = su2v[:, :, 1]
            u2 = su2v[:, :, 0]
            nc.vector.tensor_tensor(s2, y2, rD2v, ALU.mult)
            nc.vector.tensor_tensor(u2, s2, t12v, ALU.mult)

            pB2 = wk.tile([P, 36 * w], F16, tag="mv", name="pB2")
            pB2v = pB2[:].rearrange("p (e m i j w) -> p e m i j w",
                                    e=2, m=2, i=3, j=3)
            # GT element [m, first=i, second=j] natural order; contract j
            for m_ in range(2):
                nc.vector.tensor_tensor(
                    pB2v[:, :, m_],
                    GTv[:, m_].unsqueeze(1).broadcast_to([P, 2, 3, 3, w]),
                    su2v[:, :, m_].unsqueeze(2).broadcast_to([P, 2, 3, 3, w]),
                    ALU.mult)
            vt = wk.tile([P, 12 * w], F16, tag="at", name="vt")
            vtv = vt[:].rearrange("p (e m i w) -> p e m i w", e=2, m=2, i=3)
            nc.vector.tensor_tensor(vtv, pB2v[:, :, :, :, 0],
                                    pB2v[:, :, :, :, 1], ALU.add)
            v2 = wk.tile([P, 12 * w], F16, tag="v2")
            v2v = v2[:].rearrange("p (e m i w) -> p e m i w", e=2, m=2, i=3)
            nc.vector.tensor_tensor(v2v, vtv, pB2v[:, :, :, :, 2], ALU.add)
            t2 = wk.tile([P, 6 * w], F16, tag="t2")
            t2v = t2[:].rearrange("p (e i w) -> p e i w", e=2, i=3)
            nc.vector.tensor_tensor(t2v, v2v[:, :, 1], v2v[:, :, 0],
                                    ALU.subtract)
            lg2 = wk.tile([P, 6 * w], F16, tag="lg2")
            nc.vector.tensor_tensor(lg2[:], t12[:], t2[:], ALU.add)
            lgE = lg2[:, 0:3 * w]
            lgR = lg2[:, 3 * w:]

            # ---- L_phy -------------------------------------------------
            q1 = wk.tile([P, 3 * w], F16, tag="q1")
            nc.vector.tensor_tensor(q1[:], lgE, lgR, ALU.subtract)
            q2 = wk.tile([P, 3 * w], F16, tag="q2")
            nc.gpsimd.tensor_tensor(q2[:], lnP[:, 0:3 * w], lnP[:, 3 * w:],
                                    ALU.subtract)
            dls = wk.tile([P, w], F16, tag="dls")
            nc.gpsimd.tensor_tensor(dls[:], lnsE, lnsR, ALU.subtract)
            r0 = wk.tile([P, 3 * w], F16, tag="r0")
            nc.vector.tensor_tensor(r0[:], q1[:], q2[:], ALU.add)
            rphy = wk.tile([P, 3 * w], F16, tag="rphy")
            rphyv = rphy[:].rearrange("p (c w) -> p c w", c=3)
            nc.vector.tensor_tensor(
                rphyv, r0[:].rearrange("p (c w) -> p c w", c=3),
                dls[:].unsqueeze(1).broadcast_to([P, 3, w]), ALU.subtract)

            # ---- base = ln xE + lgE ------------------------------------
            lnxE = wk.tile([P, 3 * w], F16, tag="lnxE")
            lnxEv = lnxE[:].rearrange("p (c w) -> p c w", c=3)
            nc.gpsimd.tensor_tensor(
                lnxEv, lnPv[:, 0:3],
                lnsE.unsqueeze(1).broadcast_to([P, 3, w]), ALU.subtract)
            base = wk.tile([P, 3 * w], F16, tag="base")
            nc.gpsimd.tensor_tensor(base[:], lnxE[:], lgE, ALU.add)

            # ---- TPD tail ----------------------------------------------
            t1w = wk.tile([P, 12 * w], F16, tag="t1w")
            t1wv = t1w[:].rearrange("p (t i w) -> p t i w", t=4, i=3)
            nc.vector.tensor_tensor(t1wv, Aw, rDwv, ALU.mult)
            m1 = wk.tile([P, 12 * w], F16, tag="mA", name="m1")
            m1v = m1[:].rearrange("p (t k w) -> p t k w", t=4, k=3)
            baseb = base[:].rearrange("p (k w) -> p k w", k=3)                 .unsqueeze(1).broadcast_to([P, 4, 3, w])
            nc.gpsimd.tensor_tensor(m1v, lnwcv, baseb, ALU.subtract)
            m2 = wk.tile([P, 12 * w], F16, tag="mB", name="m2")
            m2v = m2[:].rearrange("p (t k w) -> p t k w", t=4, k=3)
            lnswb = lnswv.unsqueeze(2).broadcast_to([P, 4, 3, w])
            nc.vector.tensor_tensor(m2v, m1v, lnswb, ALU.subtract)
            kk = wk.tile([P, 12 * w], F16, tag="mA", name="kk")
            nc.vector.tensor_tensor(kk[:], m2[:], t1w[:], ALU.add)
            wpk = wk.tile([P, 12 * w], F16, tag="mB", name="wpk")
            wpkv = wpk[:].rearrange("p (t k w) -> p t k w", t=4, k=3)
            nc.vector.tensor_tensor(wpkv, wc4v,
                                    kk[:].rearrange("p (t k w) -> p t k w",
                                                    t=4, k=3), ALU.mult)
            tst = wk.tile([P, 4 * w], F16, tag="swt", name="tst")
            tstv = tst[:].rearrange("p (t w) -> p t w", t=4)
            nc.gpsimd.tensor_tensor(tstv, wpkv[:, :, 0], wpkv[:, :, 1],
                                    ALU.add)
            tsum = wk.tile([P, 4 * w], F16, tag="tsum")
            tsumv = tsum[:].rearrange("p (t w) -> p t w", t=4)
            nc.gpsimd.tensor_tensor(tsumv, tstv, wpkv[:, :, 2], ALU.add)
            tpd4 = wk.tile([P, 4 * w], F16, tag="tpd4")
            nc.vector.tensor_tensor(tpd4[:], tsum[:], rsw[:], ALU.mult)

            # ---- tail ACT group: squares + relu accumulators ------------
            junk6 = wk.tile([P, 6 * w], F16, tag="junk6")
            nc.scalar.activation(junk6[:], dsup[:], AF.Square,
                                 accum_out=acc[:, 0:1])
            junk3 = wk.tile([P, 6 * w], F16, tag="junk6", name="junk3")
            nc.scalar.activation(junk3[:, 0:3 * w], rphy[:], AF.Square,
                                 accum_out=acc[:, 1:2])
            junkt = wk.tile([P, 6 * w], F16, tag="junk6", name="junkt")
            nc.scalar.activation(junkt[:, 0:4 * w], tpd4[:], AF.Relu,
                                 scale=-1.0, accum_out=acc[:, 2:3])

            nc.sync.dma_start(outv[r_ * nt + it], acc[:])


_CACHED_NC = None


def _get_nc():
    global _CACHED_NC
    if _CACHED_NC is None:
        _CACHED_NC = _build()
    return _CACHED_NC


# ---------------------------------------------------------------------------
# host-side packing
# ---------------------------------------------------------------------------

def _shard_inputs(pred, target, T, g, dirs, noise):
    in_maps = []
    for c in range(NCORE):
        sl = slice(c * NPC, (c + 1) * NPC)
        pr = pred[sl].reshape(NT, P, W, 6).transpose(0, 1, 3, 2)
        tg = target[sl].reshape(NT, P, W, 6).transpose(0, 1, 3, 2)
        Ts = T[sl].reshape(NT, P, 1, W)
        gs = g[sl].reshape(NT, P, W, 9).transpose(0, 1, 3, 2)
        ns = noise[:, sl].reshape(4, NT, P, W, 3).transpose(1, 2, 0, 4, 3)             .reshape(NT, P, 12, W)
        X = np.concatenate([pr, tg, Ts, gs, ns], axis=2).astype(np.float16)
        in_maps.append({"X": np.ascontiguousarray(X.reshape(NT, P, NPLANE * W))})
    return in_maps


# ---------------------------------------------------------------------------
# numpy reference for the host-side tail (float64)
# ---------------------------------------------------------------------------

def _renorm3_np(x):
    x = np.maximum(x, 0.0)
    return x / np.maximum(x.sum(-1, keepdims=True), EPS)


def _ln_gamma_np(x, T, g):
    x = np.maximum(x, 0.0)
    Tc = np.maximum(T, 1.0)
    tau = np.clip(g / (R_GAS * np.maximum(Tc, EPS))[:, None, None], -10.0, 10.0)
    G = np.exp(-ALPHA * tau)
    denom = np.maximum(np.einsum("bj,bji->bi", x, G), EPS)
    A = np.einsum("bj,bji->bi", x, tau * G)
    term1 = A / denom
    Wm = x[:, None, :] * G / denom[:, None, :]
    inside = tau - (A / denom)[:, None, :]
    term2 = (Wm * inside).sum(-1)
    return np.clip(term1 + term2, -LN_CLIP, LN_CLIP)


def _tail_sums(pred, target, T, g, dirs, noise):
    """Raw sums (not means) of each term over the tail slice, float64."""
    pred = pred.astype(np.float64)
    target = target.astype(np.float64)
    T = T.astype(np.float64)
    g = g.astype(np.float64)
    dirs = dirs.astype(np.float64)
    noise = noise.astype(np.float64)

    sup = ((pred - target) ** 2).sum()
    xE = _renorm3_np(pred[:, :3])
    xR = _renorm3_np(pred[:, 3:])
    lgE = _ln_gamma_np(xE, T, g)
    lgR = _ln_gamma_np(xR, T, g)
    r = np.log(np.maximum(xE, EPS)) + lgE - (np.log(np.maximum(xR, EPS)) + lgR)
    phy = (r ** 2).sum()

    gd2 = 0.0
    for d in range(dirs.shape[0]):
        xp = _renorm3_np(xE + EPS_FD * dirs[d])
        xm = _renorm3_np(xE - EPS_FD * dirs[d])
        dln = (_ln_gamma_np(xp, T, g) - _ln_gamma_np(xm, T, g)) / (2 * EPS_FD)
        gd = (xE * dln).sum(-1)
        gd2 += (gd * gd).sum()

    tpd_s = 0.0
    for t_ in range(noise.shape[0]):
        wv = _renorm3_np(xE + noise[t_])
        lgw = _ln_gamma_np(wv, T, g)
        tpd = (wv * (np.log(np.maximum(wv, EPS)) + lgw
                     - np.log(np.maximum(xE, EPS)) - lgE)).sum(-1)
        tpd_s += np.maximum(MARGIN - tpd, 0.0).sum()

    return sup, phy, gd2, tpd_s


# ---------------------------------------------------------------------------
# public entry point
# ---------------------------------------------------------------------------

def _combine(results, pred, target, T, g, dirs, noise):
    parts = np.stack([r["partial"] for r in results]).astype(np.float64)
    dev = parts.sum(axis=(0, 1, 2))  # [NACC]
    sup_s = dev[0]
    phy_s = dev[1]
    tpd_s = dev[2]
    gd2_s = 0.0

    if NDEV < B:
        sl = slice(NDEV, B)
        ts, tp, tg_, tt = _tail_sums(pred[sl], target[sl], T[sl], g[sl],
                                     dirs[:, sl], noise[:, sl])
        sup_s += ts
        phy_s += tp
        gd2_s += tg_
        tpd_s += tt

    L = (sup_s / (6 * B)
         + LAM_PHY * phy_s / (3 * B)
         + LAM_GD * gd2_s / (N_DIR * B)
         + LAM_TPD * tpd_s / (N_TRIAL * B))
    return np.float32(L)


def kernel(pred, target, T, g, dirs, noise):
    nc = _get_nc()
    in_maps = _shard_inputs(pred, target, T, g, dirs, noise)
    res = run_bass_kernel_spmd(nc, in_maps, core_ids=list(range(NCORE)))
    return _combine(res.results, pred, target, T, g, dirs, noise)
